# revision 18
# baseline (speedup 1.0000x reference)
"""RGCN message-scoring kernel for Trainium2 (8 NeuronCores, SPMD).

Strategy (sharding_hint: partition graphs across devices):
- 250 graphs of exactly 200 nodes / 3200 within-graph edges are split across
  8 cores ([32,32,31,...,31], padded with zero "dummy" graphs to 32 each).
- The per-graph normalized adjacency operator B[src, (rel,dst)] is stored in
  fp8e4 as C * e4m3(1/cnt) (exact: C is a small integer count and the
  quantized reciprocal scales by powers of two), laid out src-folded
  [128, 2, KEYS] so a single DoubleRow fp8 matmul contracts all 200 srcs at
  0.5 cycles/row -- 4x the bf16 baseline's tensor throughput on the two big
  B-streaming passes (layer-1 aggregation T1 and layer-2 scoring).
- x and psi stream as hi/lo fp8 pairs (v = fp8(v) + fp8(v - fp8(v)), ~7
  effective mantissa bits, better than bf16) accumulating into the same
  PSUM tile, so precision is preserved while keeping DoubleRow rates.
- Layer-1's mean-normalization rides inside B8 (quantized); layer-2 scoring
  computes per-relation partials at PSUM rows {0,32,64,96} (M=32 matmuls
  with zero-padded psi columns -- the PE can only target 32-aligned PSUM
  partitions for narrow outputs), compacts them to an [8,200] tile with two
  selection matmuls, and applies the exact f32 correction n/e4m3(n) there,
  cancelling the layer-2 norm quantization error entirely.
- a1 transform (sum_r W1_r^T T1_r + root1^T x^T) stays bf16 (fp8 T1' fails
  the accuracy budget), as does the message MLP.
- B arrives in blocks of 2 graphs (2 DMAs per block, triple buffered);
  drains are spread across DVE/ACT/Pool to keep every engine under the PE
  critical path.
"""

import numpy as np

NG_FULL = 250       # total graphs
NPG = 200           # nodes per graph
EPG = 3200          # edges per graph
R = 8               # relations
F = 128             # feature/embedding width
G = 32              # graphs per core (padded)
NCORES = 8
KEYS = R * NPG      # 1600, relation-major: k = r*200 + dst_local
BL = 2              # graphs per DMA block
C1 = 72             # rows in second src slice (200 - 128)

_COMPILED = {}


def _bf16(a):
    import ml_dtypes
    return np.ascontiguousarray(np.asarray(a, np.float32).astype(ml_dtypes.bfloat16))


def _f8(a):
    import ml_dtypes
    return np.ascontiguousarray(np.asarray(a, np.float32).astype(ml_dtypes.float8_e4m3))


def _emit(nc, tc, io, ablate=None):
    """Emit the full per-core program body (input loads + compute + store)."""
    import concourse.mybir as mybir
    dt = mybir.dt
    AF = mybir.ActivationFunctionType
    DR = mybir.MatmulPerfMode.DoubleRow
    MUL = mybir.AluOpType.mult
    SUB = mybir.AluOpType.subtract
    T = io.__getitem__

    NPAIR = G // 2
    PAIRS_PER_BLK = BL // 2
    NBLK = G // BL
    with (
        tc.tile_pool(name="const", bufs=1) as const,
        tc.tile_pool(name="bpool", bufs=3) as bpool,
        tc.tile_pool(name="xpool", bufs=1) as xpool,
        tc.tile_pool(name="t1pool", bufs=2) as t1pool,
        tc.tile_pool(name="hpool", bufs=2) as hpool,
    ):
        # x hi/lo fp8 per block (separate tags so block b's matmuls only wait
        # on block b's DMA)
        xh_blk, xl_blk = [], []
        for b in range(NBLK):
            xh = xpool.tile([128, BL * 2 * F], dt.float8e4, tag=f"xh{b}",
                            name=f"xh{b}")
            xl = xpool.tile([128, BL * 2 * F], dt.float8e4, tag=f"xl{b}",
                            name=f"xl{b}")
            xh_blk.append(xh)
            xl_blk.append(xl)
        bc_of = {}
        bc_bufs = []

        def load_block(blk):
            bc8 = bpool.tile([128, BL * 2 * KEYS], dt.float8e4, tag="bc8")
            if len(bc_bufs) < 3:
                # first use of this physical buffer: zero the pad rows of
                # src-slice 1 (they multiply zero x rows, but NaN*0 = NaN).
                # Engine partition bases must be 32-aligned, so zero from 64
                # (rows 64..71 are rewritten by the BC8B DMA below).
                bv = bc8[:].rearrange("p (b s k) -> p b s k", b=BL, s=2)
                nc.gpsimd.memset(bv[64:128, :, 1, :], 0.0)
                bc_bufs.append(bc8)
            bv = bc8[:].rearrange("p (b s k) -> p b s k", b=BL, s=2)
            nc.sync.dma_start(
                bv[:, :, 0, :],
                T("BC8A").ap()[:, blk * BL * KEYS:(blk + 1) * BL * KEYS]
                .rearrange("p (b k) -> p b k", b=BL))
            nc.sync.dma_start(
                bv[0:C1, :, 1, :],
                T("BC8B").ap()[:, blk * BL * KEYS:(blk + 1) * BL * KEYS]
                .rearrange("p (b k) -> p b k", b=BL))
            nc.sync.dma_start(
                xh_blk[blk][:],
                T("XH").ap()[:, blk * BL * 2 * F:(blk + 1) * BL * 2 * F])
            nc.sync.dma_start(
                xl_blk[blk][:],
                T("XL").ap()[:, blk * BL * 2 * F:(blk + 1) * BL * 2 * F])
            bc_of[blk] = bc8

        load_block(0)

        w2mB = const.tile([128, 9 * G], dt.bfloat16)
        b2m = const.tile([1, G], dt.float32)
        out_sb = const.tile([1, G * NPG], dt.float32)
        w2t = const.tile([128, R * 128], dt.bfloat16)
        root2t = const.tile([128, 128], dt.bfloat16)
        b2 = const.tile([128, 1], dt.bfloat16)
        nc.sync.dma_start(w2t[:], T("W2T").ap()[:])
        nc.sync.dma_start(root2t[:], T("ROOT2T").ap()[:])
        nc.sync.dma_start(b2[:], T("B2").ap()[:])
        _emit_message(nc, tc, io, w2mB, b2m, w2t, root2t, b2)

        xt = const.tile([128, G * NPG], dt.bfloat16)
        w1l = const.tile([128, R * 128], dt.bfloat16)
        root1 = const.tile([128, 128], dt.bfloat16)
        b1 = const.tile([128, 1], dt.float32)
        norm8 = const.tile([8, G * NPG], dt.float32)
        ones8 = const.tile([8, 1], dt.bfloat16)
        nc.sync.dma_start(xt[:], T("XT").ap()[:])
        nc.sync.dma_start(w1l[:], T("W1L").ap()[:])
        nc.sync.dma_start(root1[:], T("ROOT1").ap()[:])
        nc.sync.dma_start(b1[:], T("B1").ap()[:])
        nc.sync.dma_start(norm8[:], T("NORM8").ap()[:])
        nc.vector.memset(ones8[:], 1.0)

        # psi staging: [p, s, r, c] with psi_r at col r*32, zeros elsewhere so
        # the M=32 score matmuls produce zero rows between relation rows
        psiH_bufs, psiL_bufs = [], []
        for i in range(2):
            pH = hpool.tile([128, 2 * 256], dt.float8e4, tag="psiH")
            pL = hpool.tile([128, 2 * 256], dt.float8e4, tag="psiL")
            nc.gpsimd.memset(pH[:], 0.0)
            nc.gpsimd.memset(pL[:], 0.0)
            psiH_bufs.append(pH)
            psiL_bufs.append(pL)

        with (
            tc.tile_pool(name="pst1", bufs=3, space="PSUM") as pst1,
            tc.tile_pool(name="psa", bufs=1, space="PSUM") as psa,
            tc.tile_pool(name="pssc", bufs=2, space="PSUM") as pssc,
            tc.tile_pool(name="psc8", bufs=2, space="PSUM") as psc8,
        ):
            def emit_t1(p):
                blk = p // PAIRS_PER_BLK
                bc8 = bc_of[blk]
                bv = bc8[:].rearrange("p (b s k) -> p b s k", b=BL, s=2)
                pair = p % PAIRS_PER_BLK
                t1sb = t1pool.tile([128, R * 2 * NPG], dt.bfloat16)
                t1v = t1sb[:].rearrange("p (r t k) -> p r t k", r=R, t=2)
                for pi in range(2):
                    gi = pair * 2 + pi               # graph within block
                    g = blk * BL + gi
                    xh = xh_blk[blk][:].rearrange(
                        "p (b s f) -> p b s f", b=BL, s=2)[:, gi, :, :]
                    xl = xl_blk[blk][:].rearrange(
                        "p (b s f) -> p b s f", b=BL, s=2)[:, gi, :, :]
                    for q in range(4):
                        t1p = pst1.tile([128, 400], dt.float32)
                        base = q * 400
                        for c0, cn in ((0, 256), (256, 144)):
                            rhs = bv[:, gi, :, base + c0:base + c0 + cn]
                            nc.tensor.matmul(t1p[:, c0:c0 + cn], xh, rhs,
                                             start=True, stop=False,
                                             perf_mode=DR)
                            nc.tensor.matmul(t1p[:, c0:c0 + cn], xl, rhs,
                                             start=False, stop=True,
                                             perf_mode=DR)
                        dst = t1v[:, q * 2:q * 2 + 2, pi, :]
                        src = t1p[:].rearrange("p (q k) -> p q k", q=2)
                        if q % 2 == 0:
                            nc.vector.tensor_copy(dst, src)
                        else:
                            nc.scalar.activation(dst, src, AF.Copy)
                return t1sb

            def emit_a1(p, t1sb):
                g0 = p * 2                            # first graph of pair
                a1 = psa.tile([128, 2 * NPG], dt.float32)
                for r in range(R):
                    nc.tensor.matmul(a1[:], w1l[:, r * 128:(r + 1) * 128],
                                     t1sb[:, r * 400:(r + 1) * 400],
                                     start=(r == 0), stop=False)
                nc.tensor.matmul(a1[:], root1[:],
                                 xt[:, g0 * NPG:(g0 + 2) * NPG],
                                 start=False, stop=True)
                h1 = hpool.tile([128, 2 * NPG], dt.bfloat16)
                nc.scalar.activation(h1[:], a1[:], AF.Relu, bias=b1[:])
                return h1

            def emit_score(p, h1):
                blk = p // PAIRS_PER_BLK
                bc8 = bc_of[blk]
                bv = bc8[:].rearrange("p (b s k) -> p b s k", b=BL, s=2)
                pair = p % PAIRS_PER_BLK
                for pi in range(2):
                    gi = pair * 2 + pi
                    g = blk * BL + gi
                    hoff = pi * NPG
                    # packed score psum: scA [*,0:200], scB [*,200:400],
                    # psi [*,400:418] -- one bank per buffer
                    sct = pssc.tile([128, 512], dt.float32)
                    psi_p = sct[:, 400:418]
                    w2m8 = w2mB[:].rearrange("p (n g) -> p n g", g=G)[:, 0:8, g]
                    nc.tensor.matmul(psi_p[:, 0:8], h1[:, hoff:hoff + 128],
                                     w2m8, start=True, stop=True)
                    nc.tensor.matmul(psi_p[:C1, 9:17],
                                     h1[:, hoff + 128:hoff + NPG],
                                     w2m8, start=True, stop=True)
                    # drain to hi/lo fp8 at col r*32 of the padded staging
                    pH = psiH_bufs[(p * 2 + pi) % 2]
                    pL = psiL_bufs[(p * 2 + pi) % 2]
                    pHv = pH[:].rearrange("p (s r c) -> p s r c", s=2, r=8)
                    pLv = pL[:].rearrange("p (s r c) -> p s r c", s=2, r=8)
                    nc.scalar.activation(pHv[:, 0, :, 0], psi_p[:, 0:8],
                                         AF.Copy)
                    nc.scalar.activation(pHv[0:C1, 1, :, 0], psi_p[:C1, 9:17],
                                         AF.Copy)
                    nc.vector.scalar_tensor_tensor(
                        pLv[:, 0, :, 0], psi_p[:, 0:8], 1.0,
                        pHv[:, 0, :, 0], MUL, SUB)
                    nc.vector.scalar_tensor_tensor(
                        pLv[0:C1, 1, :, 0], psi_p[:C1, 9:17], 1.0,
                        pHv[0:C1, 1, :, 0], MUL, SUB)

                    # per-relation partials: all 16 matmuls accumulate into
                    # ONE [32, 200] base-0 psum region. The lhsT window for
                    # relation r starts at flat col 31*r, putting psi_r (at
                    # flat col 32*r) at window position r -> output ROW r;
                    # every other window column is zero padding, so rows != r
                    # contribute nothing.
                    pHf = pH[:].rearrange("p (s c) -> p s c", s=2)
                    pLf = pL[:].rearrange("p (s c) -> p s c", s=2)
                    sc32 = sct[0:32, 0:NPG]
                    for r in range(R):
                        rhs = bv[:, gi, :, r * NPG:(r + 1) * NPG]
                        w0 = 31 * r
                        nc.tensor.matmul(sc32, pHf[:, :, w0:w0 + 32],
                                         rhs, start=(r == 0), stop=False,
                                         perf_mode=DR)
                        nc.tensor.matmul(sc32, pLf[:, :, w0:w0 + 32],
                                         rhs, start=False, stop=(r == R - 1),
                                         perf_mode=DR)
                    sc8n = hpool.tile([8, NPG], dt.bfloat16, tag="sc8n")
                    nc.vector.tensor_tensor(
                        sc8n[:], sct[0:8, 0:NPG],
                        norm8[:, g * NPG:(g + 1) * NPG], MUL)
                    # sum over relations + root term + bias
                    fin = psc8.tile([1, NPG], dt.float32)
                    nc.tensor.matmul(fin, ones8[:], sc8n[:],
                                     start=True, stop=False)
                    nc.tensor.matmul(fin, w2mB[:, 8 * G + g:8 * G + g + 1],
                                     h1[:, hoff:hoff + NPG],
                                     start=False, stop=True)
                    nc.scalar.activation(out_sb[0:1, g * NPG:(g + 1) * NPG],
                                         fin, AF.Identity,
                                         bias=b2m[0:1, g:g + 1])

            next_blk = 1
            prev = None
            for p in range(NPAIR):
                cur_blk = p // PAIRS_PER_BLK
                while next_blk < min(NBLK, cur_blk + 3):
                    load_block(next_blk)
                    next_blk += 1
                if prev is not None:
                    h1 = emit_a1(p - 1, prev)
                t1sb = emit_t1(p)
                if prev is not None:
                    emit_score(p - 1, h1)
                prev = t1sb
            if prev is not None:
                h1 = emit_a1(NPAIR - 1, prev)
                emit_score(NPAIR - 1, h1)

        nc.sync.dma_start(T("OUT").ap()[:], out_sb[:])


def _emit_message(nc, tc, io, w2mB, b2m, w2t, root2t, b2):
    """Message MLP + collapse of layer-2 weights against each graph's
    message vector: w2mB[:, n*G+g] = W2_n m_g (n<8) / root2 m_g (n==8),
    b2m[g] = b2 . m_g."""
    import concourse.mybir as mybir
    dt = mybir.dt
    AF = mybir.ActivationFunctionType
    T = io.__getitem__

    with (
        tc.tile_pool(name="msg", bufs=1) as msg,
        tc.tile_pool(name="psm", bufs=2, space="PSUM") as psm,
    ):
        embl = msg.tile([128, 8 * 128], dt.bfloat16)
        sel = msg.tile([128, 8 * G], dt.bfloat16)
        cont = msg.tile([1, G], dt.bfloat16)
        contw = msg.tile([1, 128], dt.bfloat16)
        contb = msg.tile([128, 1], dt.float32)
        msgw = msg.tile([128, 2 * 128], dt.bfloat16)
        msgb = msg.tile([128, 1], dt.float32)
        nc.sync.dma_start(embl[:], T("EMBL").ap()[:])
        nc.sync.dma_start(sel[:], T("SEL").ap()[:])
        nc.sync.dma_start(cont[:], T("CONT").ap()[:])
        nc.sync.dma_start(contw[:], T("CONTW").ap()[:])
        nc.sync.dma_start(contb[:], T("CONTB").ap()[:])
        nc.sync.dma_start(msgw[:], T("MSGW").ap()[:])
        nc.sync.dma_start(msgb[:], T("MSGB").ap()[:])

        # disc_embT [128f, G] = sum_c EMBL_c^T @ SEL_c
        ps_d = psm.tile([128, G], dt.float32)
        for c in range(8):
            nc.tensor.matmul(
                ps_d[:], embl[:, c * 128:(c + 1) * 128], sel[:, c * G:(c + 1) * G],
                start=(c == 0), stop=(c == 7))
        discT = msg.tile([128, G], dt.bfloat16)
        nc.vector.tensor_copy(discT[:], ps_d[:])

        # cont_embT [128, G] = relu(cont_w^T cont + cont_b)
        ps_c = psm.tile([128, G], dt.float32)
        nc.tensor.matmul(ps_c[:], contw[:], cont[:], start=True, stop=True)
        contT = msg.tile([128, G], dt.bfloat16)
        nc.scalar.activation(contT[:], ps_c[:], AF.Relu, bias=contb[:])

        # mT [128, G] = relu(msg_w^T [disc; cont] + msg_b)
        ps_m = psm.tile([128, G], dt.float32)
        nc.tensor.matmul(ps_m[:], msgw[:, 0:128], discT[:], start=True, stop=False)
        nc.tensor.matmul(ps_m[:], msgw[:, 128:256], contT[:], start=False, stop=True)
        mT = msg.tile([128, G], dt.bfloat16)
        nc.scalar.activation(mT[:], ps_m[:], AF.Relu, bias=msgb[:])

        # w2m[r] = W2_r m ; rootm = root2 m ; b2m = b2 . m
        for r in range(R):
            ps_w = psm.tile([128, G], dt.float32, tag="psw")
            nc.tensor.matmul(ps_w[:], w2t[:, r * 128:(r + 1) * 128], mT[:],
                             start=True, stop=True)
            nc.vector.tensor_copy(w2mB[:, r * G:(r + 1) * G], ps_w[:])
        ps_r = psm.tile([128, G], dt.float32, tag="psw")
        nc.tensor.matmul(ps_r[:], root2t[:], mT[:], start=True, stop=True)
        nc.vector.tensor_copy(w2mB[:, 8 * G:9 * G], ps_r[:])
        ps_b = psm.tile([1, G], dt.float32, tag="psw")
        nc.tensor.matmul(ps_b[:], b2[:], mT[:], start=True, stop=True)
        nc.vector.tensor_copy(b2m[:], ps_b[:])


def _declare_io(nc):
    import concourse.mybir as mybir
    dt = mybir.dt
    io = {}
    specs = [
        ("BC8A", [128, G * KEYS], dt.float8e4),
        ("BC8B", [C1, G * KEYS], dt.float8e4),
        ("XH", [128, G * 2 * F], dt.float8e4),
        ("XL", [128, G * 2 * F], dt.float8e4),
        ("XT", [128, G * NPG], dt.bfloat16),
        ("NORM8", [8, G * NPG], dt.float32),
        ("W1L", [128, R * 128], dt.bfloat16),
        ("ROOT1", [128, 128], dt.bfloat16),
        ("B1", [128, 1], dt.float32),
        ("W2T", [128, R * 128], dt.bfloat16),
        ("ROOT2T", [128, 128], dt.bfloat16),
        ("B2", [128, 1], dt.bfloat16),
        ("EMBL", [128, 8 * 128], dt.bfloat16),
        ("SEL", [128, 8 * G], dt.bfloat16),
        ("CONT", [1, G], dt.bfloat16),
        ("CONTW", [1, 128], dt.bfloat16),
        ("CONTB", [128, 1], dt.float32),
        ("MSGW", [128, 2 * 128], dt.bfloat16),
        ("MSGB", [128, 1], dt.float32),
    ]
    for name, shape, dtype in specs:
        io[name] = nc.dram_tensor(name, shape, dtype, kind="ExternalInput")
    io["OUT"] = nc.dram_tensor("OUT", [1, G * NPG], dt.float32,
                               kind="ExternalOutput")
    return io


def _build_program(loop_k=None, ablate=None, unroll=1):
    """Build the per-core program. With loop_k, the full body (including all
    input DMA) repeats loop_k*unroll times per NEFF execution -- used by the
    timing harness to measure steady-state per-execution time."""
    import concourse.bacc as bacc
    from concourse import tile

    import concourse.mybir as mybir

    nc = bacc.Bacc("TRN2", target_bir_lowering=False, debug=False)
    io = _declare_io(nc)
    with tile.TileContext(nc) as tc:
        if loop_k is None:
            _emit(nc, tc, io, ablate=ablate)
        else:
            with tc.For_i(0, loop_k, 1,
                          hint_engines=(mybir.EngineType.PE,)):
                for _ in range(unroll):
                    _emit(nc, tc, io, ablate=ablate)
    nc.compile()
    return nc


def _np_reference(message, x, edge_index, edge_type, batch, max_nodes,
                  W1, root1, b1, W2, root2, b2,
                  embed_table, cont_w, cont_b, msg_w, msg_b):
    """Pure-numpy fallback for inputs that violate the regular-structure
    assumptions (ragged batches or cross-graph edges)."""
    n_nodes, n_rel, n_graphs = x.shape[0], W1.shape[0], message.shape[0]
    src, dst = edge_index[0], edge_index[1]

    def conv(h, W, root, b):
        hW = np.einsum('nf,rfo->nro', h, W)
        m = hW[src, edge_type]
        key_dr = dst * n_rel + edge_type
        cnt = np.zeros(n_nodes * n_rel, h.dtype)
        np.add.at(cnt, key_dr, 1.0)
        nrm = 1.0 / np.maximum(cnt[key_dr], 1.0)
        agg = np.zeros((n_nodes, W.shape[2]), h.dtype)
        np.add.at(agg, dst, m * nrm[:, None])
        return agg + h @ root + b

    h = np.maximum(conv(x, W1, root1, b1), 0)
    node_emb = conv(h, W2, root2, b2)
    disc = embed_table[message[:, 0].astype(np.int32)]
    cont = np.maximum(message[:, 1:2].astype(np.float32) @ cont_w + cont_b, 0)
    mrep = np.maximum(np.concatenate([disc, cont], 1) @ msg_w + msg_b, 0)
    scores = (node_emb * mrep[batch]).sum(1)
    cnts = np.bincount(batch, minlength=n_graphs)
    start = np.cumsum(cnts) - cnts
    pos = np.arange(n_nodes) - start[batch]
    logits = np.full((n_graphs, int(max_nodes)), -np.inf, np.float32)
    ok = pos < int(max_nodes)  # jax .at[].set drops OOB indices; match that
    logits[batch[ok], pos[ok]] = scores.astype(np.float32)[ok]
    return logits


def kernel(**inputs):
    import ml_dtypes
    message = np.asarray(inputs["message"], np.float32)
    x = np.asarray(inputs["x"], np.float32)
    edge_index = np.asarray(inputs["edge_index"])
    edge_type = np.asarray(inputs["edge_type"])
    batch = np.asarray(inputs["batch"])
    max_nodes = int(np.asarray(inputs["max_nodes"]))
    W1 = np.asarray(inputs["W1"], np.float32)
    root1 = np.asarray(inputs["root1"], np.float32)
    b1 = np.asarray(inputs["b1"], np.float32)
    W2 = np.asarray(inputs["W2"], np.float32)
    root2 = np.asarray(inputs["root2"], np.float32)
    b2 = np.asarray(inputs["b2"], np.float32)
    embed_table = np.asarray(inputs["embed_table"], np.float32)
    cont_w = np.asarray(inputs["cont_w"], np.float32)
    cont_b = np.asarray(inputs["cont_b"], np.float32)
    msg_w = np.asarray(inputs["msg_w"], np.float32)
    msg_b = np.asarray(inputs["msg_b"], np.float32)

    ng = message.shape[0]
    src, dst = edge_index[0].astype(np.int64), edge_index[1].astype(np.int64)
    et = edge_type.astype(np.int64)

    regular = (
        ng == NG_FULL
        and x.shape == (NG_FULL * NPG, F)
        and max_nodes == NPG
        and W1.shape == (R, F, F)
        and src.shape[0] == NG_FULL * EPG
        and embed_table.shape == (1000, F)
        and np.array_equal(batch, np.repeat(np.arange(ng), NPG))
        and np.array_equal(src // NPG, np.repeat(np.arange(ng), EPG))
        and np.array_equal(dst // NPG, np.repeat(np.arange(ng), EPG))
        and et.min() >= 0 and et.max() < R
        and message[:, 0].min() >= 0 and message[:, 0].max() < 1000
    )
    if not regular:
        return _np_reference(**inputs)

    f8 = ml_dtypes.float8_e4m3
    bf16 = ml_dtypes.bfloat16

    # ---- host index preprocessing ----
    eg = dst // NPG
    dst_l = dst % NPG
    src_l = src % NPG
    key = et * NPG + dst_l                       # relation-major local key
    gk = eg * KEYS + key
    cnt = np.bincount(gk, minlength=NG_FULL * KEYS).astype(np.float32)
    n_true = 1.0 / np.maximum(cnt, 1.0)
    n_fp8 = n_true.astype(f8).astype(np.float32)  # e4m3-rounded reciprocal
    ncorr = (n_true / n_fp8).astype(np.float32)   # exact layer-2 correction
    # B8 = C * e4m3(1/cnt): duplicate edges add the same fp8 value -> still
    # exactly representable (integer times a 3-bit mantissa value)
    B = np.zeros((NG_FULL * NPG, KEYS), np.float32)
    np.add.at(B, (eg * NPG + src_l, key), n_fp8[gk])
    B = B.astype(f8).reshape(NG_FULL, NPG, KEYS)

    counts = [32, 32, 31, 31, 31, 31, 31, 31]
    starts = np.concatenate([[0], np.cumsum(counts)])[:-1]
    tok = message[:, 0].astype(np.int64)
    contv = message[:, 1]

    # x hi/lo fp8 split (hi + lo reconstructs x to ~7 mantissa bits)
    x_hi = x.astype(f8)
    x_lo = (x - x_hi.astype(np.float32)).astype(f8)

    # weights (shared across cores)
    shared = {
        "W1L": _bf16(W1.transpose(1, 0, 2).reshape(128, R * 128)),
        "ROOT1": _bf16(root1),
        "B1": b1.reshape(128, 1).astype(np.float32),
        "W2T": _bf16(W2.transpose(2, 0, 1).reshape(128, R * 128)),
        "ROOT2T": _bf16(root2.T),
        "B2": _bf16(b2.reshape(128, 1)),
        "CONTW": _bf16(cont_w),
        "CONTB": cont_b.reshape(128, 1).astype(np.float32),
        "MSGW": _bf16(msg_w.reshape(2, 128, 128).transpose(1, 0, 2).reshape(128, 256)),
        "MSGB": msg_b.reshape(128, 1).astype(np.float32),
    }
    embl = np.zeros((1024, F), np.float32)
    embl[:1000] = embed_table
    shared["EMBL"] = _bf16(embl.reshape(8, 128, F).transpose(1, 0, 2).reshape(128, 8 * F))

    ncorr_g = ncorr.reshape(NG_FULL, R, NPG)
    in_maps = []
    for c in range(NCORES):
        g0, gc = int(starts[c]), counts[c]
        # XH/XL: [p, (g, s, f)] src-folded fp8, pad rows zero
        def fold(xa):
            xg = xa.reshape(NG_FULL, NPG, F)[g0:g0 + gc]
            v = np.zeros((128, G, 2, F), f8)
            v[:, :gc, 0, :] = xg[:, 0:128].transpose(1, 0, 2)
            v[:C1, :gc, 1, :] = xg[:, 128:NPG].transpose(1, 0, 2)
            return np.ascontiguousarray(v.reshape(128, G * 2 * F))
        xh_v = fold(x_hi)
        xl_v = fold(x_lo)
        # XT: x^T bf16
        xg = x.astype(bf16).reshape(NG_FULL, NPG, F)[g0:g0 + gc]
        xtv = np.zeros((128, G * NPG), bf16)
        xtv[:, :gc * NPG] = xg.reshape(gc * NPG, F).T
        # B chunks [128|72, g*KEYS + k] fp8
        Bg = B[g0:g0 + gc]                       # [gc, 200, 1600]
        bca = np.zeros((128, G * KEYS), f8)
        bca[:, :gc * KEYS] = Bg[:, 0:128].transpose(1, 0, 2).reshape(128, gc * KEYS)
        bcb = np.zeros((C1, G * KEYS), f8)
        bcb[:, :gc * KEYS] = Bg[:, 128:NPG].transpose(1, 0, 2).reshape(C1, gc * KEYS)
        # NORM8 [8, g*NPG + d] f32 (1.0 on pad graphs)
        nrm = np.ones((8, G * NPG), np.float32)
        nrm[:, :gc * NPG] = ncorr_g[g0:g0 + gc].transpose(1, 0, 2).reshape(R, gc * NPG)
        # message-side inputs
        selv = np.zeros((1024, G), np.float32)
        selv[tok[g0:g0 + gc], np.arange(gc)] = 1.0
        sel = _bf16(selv.reshape(8, 128, G).transpose(1, 0, 2).reshape(128, 8 * G))
        cont_row = np.zeros((1, G), np.float32)
        cont_row[0, :gc] = contv[g0:g0 + gc]

        m = dict(shared)
        m.update({
            "XH": xh_v, "XL": xl_v, "XT": xtv,
            "BC8A": bca, "BC8B": bcb, "NORM8": nrm,
            "SEL": sel, "CONT": _bf16(cont_row),
        })
        in_maps.append(m)

    from concourse.bass_utils import run_bass_kernel_spmd
    if "nc" not in _COMPILED:
        _COMPILED["nc"] = _build_program()
    global _LAST_IN_MAPS
    _LAST_IN_MAPS = in_maps
    res = run_bass_kernel_spmd(_COMPILED["nc"], in_maps, core_ids=list(range(NCORES)))

    out = np.empty((NG_FULL, NPG), np.float32)
    for c in range(NCORES):
        g0, gc = int(starts[c]), counts[c]
        out[g0:g0 + gc] = res.results[c]["OUT"].reshape(G, NPG)[:gc]
    return out
